# revision 6
# baseline (speedup 1.0000x reference)
"""CrossLayerTranscoder Trainium2 kernel.

Shards the d_transcoder (feature) axis across 8 NeuronCores (768 features
per layer per core).  Each core encodes its feature shard for all 6 layers
(acts kept feature-major on-chip), then decodes partial reconstructions for
every layer j accumulating over source layers i <= j.  The feature-shard
all-reduce is done on the host when unsharding (partials summed + b_dec).

v2 design (from baseline trace analysis):
  * Everything is statically resident in SBUF (~149 KB/partition) -- no
    tile-pool buffer reuse, so the DMA rings never stall waiting on
    compute and the PE never waits on ring head-of-line blocking.
  * DMA issue order == PE consumption order, split across the two HWDGE
    rings (sync/scalar); x, b_enc and the output stores ride the gpsimd
    SWDGE queue so the rings carry only weights.
  * W_dec transfers are fused into runs of <=3 pairs (6912B rows) for
    better per-queue throughput; W_enc rides per-layer (4608B rows).
  * Encode and decode are interleaved in emission order so the PE is
    never starved: enc0 enc1 dec_j0 enc2 dec_j1 ... enc5 dec_j4 dec_j5.
  * PSUM: 3 encode tiles [128,384] + 4 decode tiles [128,384] = 7 banks;
    deep enough that the PE never waits on the DVE relu/copy drain.
"""

import numpy as np

import concourse.bass as bass
import concourse.mybir as mybir
from concourse.bass import ts
from concourse.tile import TileContext
from concourse.bass_utils import run_bass_kernel_spmd

L = 6            # layers
T = 128          # tokens
D = 768          # d_model
DT = 6144        # d_transcoder
N_CORES = 8
F = DT // N_CORES   # features per layer per core = 768
KD = D // 128       # d_model chunks of 128 = 6
KF = F // 128       # feature chunks of 128 = 6
# decode pairs in j-outer order (only upper triangle j >= i is nonzero)
PAIRS = [(i, j) for j in range(L) for i in range(j + 1)]
NPAIRS = len(PAIRS)
PAIR_IDX = {p: n for n, p in enumerate(PAIRS)}
# first pair index of each j-group and the <=3-pair DMA runs per group
JSTART = [PAIR_IDX[(0, j)] for j in range(L)] + [NPAIRS]
WD_RUNS = []  # list of (n0, n1) runs, grouped by j
for j in range(L):
    n0, n1 = JSTART[j], JSTART[j + 1]
    runs = []
    while n0 < n1:
        step = min(3, n1 - n0)
        runs.append((n0, n0 + step))
        n0 += step
    WD_RUNS.append(runs)

F32 = mybir.dt.float32
BF16 = mybir.dt.bfloat16
U8 = mybir.dt.uint8
FP8E3 = mybir.dt.float8e3

# W_enc/W_dec ride as fp8 e3m4 (1-3-4, bias 3, max normal 15.5), pre-scaled
# by WD_SCALE on the host.  S=192 keeps both the denormal mass and the clip
# mass negligible (HW-verified: PE preserves e3m4 denormals; measured err
# tracks the ml_dtypes sim).  b_enc is pre-scaled by S so acts carry the
# encode x S factor in bf16; the decode PSUM->SBUF copy unscales by 1/S^2.
# End-to-end rel-err ~1.8e-2 vs the 2e-2 gate.
WD_SCALE = 192.0
WD_MAXN = 15.5


def _split_multiwaits(nc):
    """This container's walrus rejects >1 sync-wait per instruction; split
    extra waits onto same-engine NOPs inserted immediately before."""
    for fn in nc.m.functions:
        for bb in fn.blocks:
            new = []
            for ins in bb.instructions:
                si = ins.sync_info
                if si is not None and si.on_wait and len(si.on_wait) > 1:
                    waits = list(si.on_wait)
                    for w in waits[:-1]:
                        nop = mybir.InstNoOp(
                            name=nc.get_next_instruction_name(),
                            engine=ins.engine,
                            ins=[],
                            outs=[],
                            sync_info=mybir.SyncInfo(on_wait=[w], on_update=[]),
                        )
                        new.append(nop)
                    ins.sync_info = mybir.SyncInfo(
                        on_wait=[waits[-1]], on_update=list(si.on_update or [])
                    )
                new.append(ins)
            bb.instructions = new


def _build_nc():
    nc = bass.Bass()
    # all DRAM layouts are partition(128)-major so fused DMAs get long
    # contiguous per-partition rows
    xt_d = nc.dram_tensor("xt", [128, L, KD, T], BF16, kind="ExternalInput")
    we_d = nc.dram_tensor("we", [128, L, KF, KD, 128], U8, kind="ExternalInput")
    wd_d = nc.dram_tensor("wd", [128, 2, NPAIRS, KF, 384], U8, kind="ExternalInput")
    be_d = nc.dram_tensor("be", [128, L, KF], F32, kind="ExternalInput")
    out_d = nc.dram_tensor("out", [L, 128, D], BF16, kind="ExternalOutput")

    with TileContext(nc) as tc:
        with (
            tc.tile_pool(name="const", bufs=1) as cpool,
            tc.tile_pool(name="pse", bufs=4, space="PSUM") as pse,
            tc.tile_pool(name="psd", bufs=4, space="PSUM") as psd,
        ):
            X = cpool.tile([128, L, KD, T], BF16, tag="x")
            WE = cpool.tile([128, L, KF, KD, 128], U8, tag="we")
            WD = cpool.tile([128, 2, NPAIRS, KF, 384], U8, tag="wd")
            BE = cpool.tile([128, L, KF], F32, tag="be")
            A = cpool.tile([128, L, KF, T], BF16, tag="acts")
            OUT = cpool.tile([128, L, D], BF16, tag="out")
            WARM = cpool.tile([128, 128], BF16, tag="warm")

            # ---- DMA issue, ordered by PE consumption time per queue ----
            # gpsimd SWDGE: bias, then x layers 1..5 (layer 0 rides a ring
            # so the first matmul can start ASAP)
            nc.gpsimd.memset(WARM[:], 0.0)
            nc.gpsimd.dma_start(out=BE[:], in_=be_d[:])
            for l in range(1, L):
                nc.gpsimd.dma_start(out=X[:, l], in_=xt_d[:, l])

            # empirically the Scalar HWDGE ring (r1) ramps faster / runs
            # hotter than the Sync ring (r0), so the startup-critical
            # encoder weights ride r1 and x/l0 rides r0.
            r0, r1 = nc.sync, nc.scalar
            r0.dma_start(out=X[:, 0], in_=xt_d[:, 0])
            r1.dma_start(out=WE[:, 0, 0:3], in_=we_d[:, 0, 0:3])
            r1.dma_start(out=WE[:, 0, 3:6], in_=we_d[:, 0, 3:6])
            r1.dma_start(out=WE[:, 1], in_=we_d[:, 1])

            def wd_runs(ring, h, j):
                for (a, b) in WD_RUNS[j]:
                    ring.dma_start(out=WD[:, h, a:b], in_=wd_d[:, h, a:b])

            wd_runs(r0, 0, 0)
            r0.dma_start(out=WD[:, 1, 0:1], in_=wd_d[:, 1, 0:1])
            r1.dma_start(out=WE[:, 2], in_=we_d[:, 2])
            wd_runs(r1, 0, 1)
            wd_runs(r0, 1, 1)
            r0.dma_start(out=WE[:, 3], in_=we_d[:, 3])
            wd_runs(r1, 0, 2)
            wd_runs(r0, 1, 2)
            r1.dma_start(out=WE[:, 4], in_=we_d[:, 4])
            wd_runs(r1, 0, 3)
            wd_runs(r0, 1, 3)
            r0.dma_start(out=WE[:, 5], in_=we_d[:, 5])
            wd_runs(r1, 0, 4)
            wd_runs(r0, 1, 4)
            wd_runs(r1, 0, 5)
            wd_runs(r0, 1, 5)

            # ---- PE warm-up: ~24 dummy matmuls on a memset tile keep the
            # tensor engine busy (and its DVFS ramping) while the first
            # weight tiles stream in; each is an independent 128-row matmul
            # into a scratch PSUM tile (a psd-pool buffer recycled by the
            # decoder long after the warm-ups retire).
            WPS = psd.tile([128, 384], F32, tag="psd")
            for _ in range(24):
                nc.tensor.matmul(WPS[:, 0:128], WARM[:], WARM[:], start=True, stop=True)

            # ---- compute, interleaved so the PE never starves ----
            def enc(l):
                # acts[f, t] = relu(W_enc^T-chunks @ x^T + b_enc)
                for ft in range(KF):
                    ps = pse.tile([128, T], F32, tag="pse")
                    for kd in range(KD):
                        nc.tensor.matmul(
                            ps[:],
                            WE[:, l, ft, kd, :].bitcast(FP8E3),
                            X[:, l, kd, :],
                            start=(kd == 0),
                            stop=(kd == KD - 1),
                        )
                    # relu(ps + b_enc) on DVE
                    nc.vector.tensor_scalar(
                        out=A[:, l, ft, :],
                        in0=ps[:],
                        scalar1=BE[:, l, ts(ft, 1)],
                        scalar2=0.0,
                        op0=mybir.AluOpType.add,
                        op1=mybir.AluOpType.max,
                    )

            def dec(j):
                # recon[j][t, d] = sum_{i<=j} acts_i^T-chunks @ W_dec[i,j]
                ps0 = psd.tile([128, 384], F32, tag="psd")
                ps1 = psd.tile([128, 384], F32, tag="psd")
                for i in range(j + 1):
                    n = PAIR_IDX[(i, j)]
                    for h, psx in ((0, ps0), (1, ps1)):
                        for kf in range(KF):
                            nc.tensor.matmul(
                                psx[:],
                                A[:, i, kf, :],
                                WD[:, h, n, kf, :].bitcast(FP8E3),
                                start=(i == 0 and kf == 0),
                                stop=(i == j and kf == KF - 1),
                            )
                # PSUM->SBUF copy with the fp8 pre-scale folded in
                for h, psx in ((0, ps0), (1, ps1)):
                    nc.vector.tensor_scalar(
                        out=OUT[:, j, ts(h, 384)], in0=psx[:],
                        scalar1=1.0 / (WD_SCALE * WD_SCALE), scalar2=None,
                        op0=mybir.AluOpType.mult,
                    )
                    if j == L - 1:
                        # final store in halves on the (drained) rings so
                        # the h0 store overlaps the h1 matmuls+copy
                        (r0 if h == 0 else r1).dma_start(
                            out=out_d[j, :, ts(h, 384)], in_=OUT[:, j, ts(h, 384)]
                        )
                if j < L - 1:
                    # off-ring store, overlapped with the remaining stream
                    nc.gpsimd.dma_start(out=out_d[j], in_=OUT[:, j, :])

            enc(0)
            enc(1)
            dec(0)
            enc(2)
            dec(1)
            enc(3)
            dec(2)
            enc(4)
            dec(3)
            enc(5)
            dec(4)
            dec(5)

    _split_multiwaits(nc)
    return nc


_NC_CACHE = {}


def _get_nc():
    if "nc" not in _NC_CACHE:
        _NC_CACHE["nc"] = _build_nc()
    return _NC_CACHE["nc"]


def _shard_inputs(x, W_enc, b_enc):
    """Host-side pre-swizzle into per-core DMA-friendly layouts."""
    import ml_dtypes

    npbf = np.dtype(ml_dtypes.bfloat16)
    e3m4 = np.dtype(ml_dtypes.float8_e3m4)
    # xt[p, l, kd, t] = x[l, t, kd*128+p] -- same on every core
    xt = np.ascontiguousarray(
        x.transpose(2, 0, 1).reshape(KD, 128, L, T).transpose(1, 2, 0, 3)
    ).astype(npbf)
    in_maps = []
    for c in range(N_CORES):
        fs = c * F
        w = np.clip(W_enc[:, fs : fs + F, :] * WD_SCALE, -WD_MAXN, WD_MAXN)
        # we[p, l, ft, kd, fi] = q(S * W_enc[l, fs+ft*128+fi, kd*128+p])
        we = (
            np.ascontiguousarray(
                w.transpose(0, 2, 1)                    # [L, D, F]
                .reshape(L, KD, 128, KF, 128)
                .transpose(2, 0, 3, 1, 4)               # [128, L, KF, KD, 128]
            )
            .astype(e3m4)
            .view(np.uint8)
        )
        # b_enc pre-scaled by S so acts carry the encode x S factor
        be = np.ascontiguousarray(
            b_enc[:, fs : fs + F].reshape(L, KF, 128).transpose(2, 0, 1)
        ).astype(np.float32) * WD_SCALE
        in_maps.append({"xt": xt, "we": we, "be": be})
    return in_maps


def _shard_wdec(W_dec):
    import ml_dtypes

    e3m4 = np.dtype(ml_dtypes.float8_e3m4)
    shards = []
    for c in range(N_CORES):
        fs = c * F
        # wd[p, h, n, kf, d] = q(S * W_dec[i_n, j_n, fs+kf*128+p, h*384+d])
        wd = np.empty((128, 2, NPAIRS, KF, 384), dtype=np.uint8)
        for n, (i, j) in enumerate(PAIRS):
            blk = np.clip(
                W_dec[i, j, fs : fs + F, :] * WD_SCALE, -WD_MAXN, WD_MAXN
            )  # [F, D]
            wd[:, :, n] = (
                blk.reshape(KF, 128, 2, 384)
                .transpose(1, 2, 0, 3)                  # [128, 2, KF, 384]
                .astype(e3m4)
                .view(np.uint8)
            )
        shards.append(wd)
    return shards


def kernel(x, W_enc, b_enc, b_dec, W_dec, dec_mask=None, **_unused):
    x = np.asarray(x, dtype=np.float32)
    W_enc = np.asarray(W_enc, dtype=np.float32)
    b_enc = np.asarray(b_enc, dtype=np.float32)
    b_dec = np.asarray(b_dec, dtype=np.float32)
    W_dec = np.asarray(W_dec, dtype=np.float32)

    nc = _get_nc()

    in_maps = _shard_inputs(x, W_enc, b_enc)
    wd_shards = _shard_wdec(W_dec)
    for c in range(N_CORES):
        in_maps[c]["wd"] = wd_shards[c]

    res = run_bass_kernel_spmd(nc, in_maps, core_ids=list(range(N_CORES)))

    # host-side all-reduce over feature shards + decoder bias
    recon = np.zeros((L, T, D), dtype=np.float32)
    for c in range(N_CORES):
        recon += np.asarray(res.results[c]["out"]).astype(np.float32)
    recon += b_dec[:, None, :]
    return recon


# revision 9
# speedup vs baseline: 1.0417x; 1.0417x over previous
"""CrossLayerTranscoder Trainium2 kernel.

Shards the d_transcoder (feature) axis across 8 NeuronCores (768 features
per layer per core).  Each core encodes its feature shard for all 6 layers
(acts kept feature-major on-chip), then decodes partial reconstructions for
every layer j accumulating over source layers i <= j.  The feature-shard
all-reduce is done on the host when unsharding (partials summed + b_dec).

v2 design (from baseline trace analysis):
  * Everything is statically resident in SBUF (~149 KB/partition) -- no
    tile-pool buffer reuse, so the DMA rings never stall waiting on
    compute and the PE never waits on ring head-of-line blocking.
  * DMA issue order == PE consumption order, split across the two HWDGE
    rings (sync/scalar); x, b_enc and the output stores ride the gpsimd
    SWDGE queue so the rings carry only weights.
  * W_dec transfers are fused into runs of <=3 pairs (6912B rows) for
    better per-queue throughput; W_enc rides per-layer (4608B rows).
  * Encode and decode are interleaved in emission order so the PE is
    never starved: enc0 enc1 dec_j0 enc2 dec_j1 ... enc5 dec_j4 dec_j5.
  * PSUM: 3 encode tiles [128,384] + 4 decode tiles [128,384] = 7 banks;
    deep enough that the PE never waits on the DVE relu/copy drain.
"""

import numpy as np

import concourse.bass as bass
import concourse.mybir as mybir
from concourse.bass import ts
from concourse.tile import TileContext
from concourse.bass_utils import run_bass_kernel_spmd

L = 6            # layers
T = 128          # tokens
D = 768          # d_model
DT = 6144        # d_transcoder
N_CORES = 8
F = DT // N_CORES   # features per layer per core = 768
KD = D // 128       # d_model chunks of 128 = 6
KF = F // 128       # feature chunks of 128 = 6
# decode pairs in j-outer order (only upper triangle j >= i is nonzero)
PAIRS = [(i, j) for j in range(L) for i in range(j + 1)]
NPAIRS = len(PAIRS)
PAIR_IDX = {p: n for n, p in enumerate(PAIRS)}
# first pair index of each j-group and the <=3-pair DMA runs per group
JSTART = [PAIR_IDX[(0, j)] for j in range(L)] + [NPAIRS]
WD_RUNS = []  # list of (n0, n1) runs, grouped by j
for j in range(L):
    n0, n1 = JSTART[j], JSTART[j + 1]
    runs = []
    while n0 < n1:
        step = min(3, n1 - n0)
        runs.append((n0, n0 + step))
        n0 += step
    WD_RUNS.append(runs)

F32 = mybir.dt.float32
BF16 = mybir.dt.bfloat16
U8 = mybir.dt.uint8
FP8E3 = mybir.dt.float8e3

# W_enc/W_dec ride as fp8 e3m4 (1-3-4, bias 3, max normal 15.5), pre-scaled
# by WD_SCALE on the host.  S=192 keeps both the denormal mass and the clip
# mass negligible (HW-verified: PE preserves e3m4 denormals; measured err
# tracks the ml_dtypes sim).  b_enc is pre-scaled by S so acts carry the
# encode x S factor in bf16; the decode PSUM->SBUF copy unscales by 1/S^2.
# End-to-end rel-err ~1.8e-2 vs the 2e-2 gate.
WD_SCALE = 192.0
WD_MAXN = 15.5


def _split_multiwaits(nc):
    """This container's walrus rejects >1 sync-wait per instruction; split
    extra waits onto same-engine NOPs inserted immediately before."""
    for fn in nc.m.functions:
        for bb in fn.blocks:
            new = []
            for ins in bb.instructions:
                si = ins.sync_info
                if si is not None and si.on_wait and len(si.on_wait) > 1:
                    waits = list(si.on_wait)
                    for w in waits[:-1]:
                        nop = mybir.InstNoOp(
                            name=nc.get_next_instruction_name(),
                            engine=ins.engine,
                            ins=[],
                            outs=[],
                            sync_info=mybir.SyncInfo(on_wait=[w], on_update=[]),
                        )
                        new.append(nop)
                    ins.sync_info = mybir.SyncInfo(
                        on_wait=[waits[-1]], on_update=list(si.on_update or [])
                    )
                new.append(ins)
            bb.instructions = new


def _build_nc():
    nc = bass.Bass()
    # all DRAM layouts are partition(128)-major so fused DMAs get long
    # contiguous per-partition rows
    xt_d = nc.dram_tensor("xt", [128, L, KD, T], BF16, kind="ExternalInput")
    we_d = nc.dram_tensor("we", [128, L, KF, KD, 128], U8, kind="ExternalInput")
    wd_d = nc.dram_tensor("wd", [128, 2, NPAIRS, KF, 384], U8, kind="ExternalInput")
    be_d = nc.dram_tensor("be", [128, L, KF], F32, kind="ExternalInput")
    out_d = nc.dram_tensor("out", [L, 128, D], BF16, kind="ExternalOutput")

    with TileContext(nc) as tc:
        with (
            tc.tile_pool(name="const", bufs=1) as cpool,
            tc.tile_pool(name="pse", bufs=4, space="PSUM") as pse,
            tc.tile_pool(name="psd", bufs=4, space="PSUM") as psd,
        ):
            X = cpool.tile([128, L, KD, T], BF16, tag="x")
            WE = cpool.tile([128, L, KF, KD, 128], U8, tag="we")
            WD = cpool.tile([128, 2, NPAIRS, KF, 384], U8, tag="wd")
            BE = cpool.tile([128, L, KF], F32, tag="be")
            A = cpool.tile([128, L, KF, T], BF16, tag="acts")
            OUT = cpool.tile([128, L, D], BF16, tag="out")
            WARM = cpool.tile([128, 128], BF16, tag="warm")

            # ---- DMA issue, ordered by PE consumption time per queue ----
            # gpsimd SWDGE: bias, then x layers 1..5 (layer 0 rides a ring
            # so the first matmul can start ASAP)
            nc.gpsimd.memset(WARM[:], 0.0)
            nc.gpsimd.dma_start(out=BE[:], in_=be_d[:])
            for l in range(1, L):
                nc.gpsimd.dma_start(out=X[:, l], in_=xt_d[:, l])

            # Loads ride the two HWDGE rings strictly in PE consumption
            # order, alternating rings to halve each queue's depth.  The
            # decode is split into two d-halves passes (all h0 pairs, then
            # all h1 pairs) so the early, bandwidth-starved window (all 8
            # cores loading at once) only has to supply half the decoder
            # weight stream; the h1 pairs arrive late when the other cores
            # have drained and per-core HBM share roughly doubles.
            r0, r1 = nc.sync, nc.scalar

            def wd_runs(ring, h, j):
                for (a, b) in WD_RUNS[j]:
                    ring.dma_start(out=WD[:, h, a:b], in_=wd_d[:, h, a:b])

            r1.dma_start(out=WE[:, 0, 0:3], in_=we_d[:, 0, 0:3])
            r0.dma_start(out=X[:, 0], in_=xt_d[:, 0])
            r1.dma_start(out=WE[:, 0, 3:6], in_=we_d[:, 0, 3:6])
            r0.dma_start(out=WE[:, 1, 0:3], in_=we_d[:, 1, 0:3])
            r1.dma_start(out=WE[:, 1, 3:6], in_=we_d[:, 1, 3:6])
            wd_runs(r0, 0, 0)
            r1.dma_start(out=WE[:, 2], in_=we_d[:, 2])
            wd_runs(r0, 0, 1)
            r1.dma_start(out=WE[:, 3], in_=we_d[:, 3])
            wd_runs(r0, 0, 2)
            r1.dma_start(out=WE[:, 4], in_=we_d[:, 4])
            wd_runs(r0, 0, 3)
            r1.dma_start(out=WE[:, 5], in_=we_d[:, 5])
            wd_runs(r1, 0, 4)
            wd_runs(r0, 0, 5)
            wd_runs(r1, 1, 0)
            wd_runs(r0, 1, 1)
            wd_runs(r0, 1, 2)
            wd_runs(r1, 1, 3)
            wd_runs(r1, 1, 4)
            wd_runs(r0, 1, 5)

            # ---- PE warm-up: dummy matmuls on a memset tile keep the
            # tensor engine busy (and its DVFS ramp alive) while the early
            # weight transfers stream in.  Deliberately sized LARGE (~5.5us)
            # so every core's real stream starts with a ~2MB prefetch lead:
            # the per-core HBM share during the all-cores-loading window
            # (~300-340 GB/s) barely matches the PE's ~330 GB/s demand, so
            # starting with a buffer converts many small mid-stream stalls
            # (each of which would reset the tensor engine's DVFS ramp)
            # into one warm wait up front.
            WPS = psd.tile([128, 384], F32, tag="psd")
            for _ in range(80):
                nc.tensor.matmul(WPS[:, 0:128], WARM[:], WARM[:], start=True, stop=True)

            # ---- compute, interleaved so the PE never starves ----
            def enc(l):
                # acts[f, t] = relu(W_enc^T-chunks @ x^T + b_enc)
                for ft in range(KF):
                    ps = pse.tile([128, T], F32, tag="pse")
                    for kd in range(KD):
                        nc.tensor.matmul(
                            ps[:],
                            WE[:, l, ft, kd, :].bitcast(FP8E3),
                            X[:, l, kd, :],
                            start=(kd == 0),
                            stop=(kd == KD - 1),
                        )
                    # relu(ps + b_enc) on DVE
                    nc.vector.tensor_scalar(
                        out=A[:, l, ft, :],
                        in0=ps[:],
                        scalar1=BE[:, l, ts(ft, 1)],
                        scalar2=0.0,
                        op0=mybir.AluOpType.add,
                        op1=mybir.AluOpType.max,
                    )

            def dec_h(j, h):
                # recon[j][t, h-half] = sum_{i<=j} acts_i^T-chunks @ W_dec
                psx = psd.tile([128, 384], F32, tag="psd")
                for i in range(j + 1):
                    n = PAIR_IDX[(i, j)]
                    for kf in range(KF):
                        nc.tensor.matmul(
                            psx[:],
                            A[:, i, kf, :],
                            WD[:, h, n, kf, :].bitcast(FP8E3),
                            start=(i == 0 and kf == 0),
                            stop=(i == j and kf == KF - 1),
                        )
                # PSUM->SBUF copy with the fp8 pre-scale folded in
                nc.vector.tensor_scalar(
                    out=OUT[:, j, ts(h, 384)], in0=psx[:],
                    scalar1=1.0 / (WD_SCALE * WD_SCALE), scalar2=None,
                    op0=mybir.AluOpType.mult,
                )
                if j == L - 1:
                    # j5 h0 rides Q0 mid-stream; the final h1 half goes on
                    # the (drained) sync ring right after its copy
                    (nc.gpsimd if h == 0 else r0).dma_start(
                        out=out_d[j, :, ts(h, 384)], in_=OUT[:, j, ts(h, 384)]
                    )
                elif h == 1:
                    # both halves done -> store, overlapped with the stream
                    nc.gpsimd.dma_start(out=out_d[j], in_=OUT[:, j, :])

            enc(0)
            enc(1)
            dec_h(0, 0)
            enc(2)
            dec_h(1, 0)
            enc(3)
            dec_h(2, 0)
            enc(4)
            dec_h(3, 0)
            enc(5)
            dec_h(4, 0)
            dec_h(5, 0)
            for j in range(L):
                dec_h(j, 1)

    _split_multiwaits(nc)
    return nc


_NC_CACHE = {}


def _get_nc():
    if "nc" not in _NC_CACHE:
        _NC_CACHE["nc"] = _build_nc()
    return _NC_CACHE["nc"]


def _shard_inputs(x, W_enc, b_enc):
    """Host-side pre-swizzle into per-core DMA-friendly layouts."""
    import ml_dtypes

    npbf = np.dtype(ml_dtypes.bfloat16)
    e3m4 = np.dtype(ml_dtypes.float8_e3m4)
    # xt[p, l, kd, t] = x[l, t, kd*128+p] -- same on every core
    xt = np.ascontiguousarray(
        x.transpose(2, 0, 1).reshape(KD, 128, L, T).transpose(1, 2, 0, 3)
    ).astype(npbf)
    in_maps = []
    for c in range(N_CORES):
        fs = c * F
        w = np.clip(W_enc[:, fs : fs + F, :] * WD_SCALE, -WD_MAXN, WD_MAXN)
        # we[p, l, ft, kd, fi] = q(S * W_enc[l, fs+ft*128+fi, kd*128+p])
        we = (
            np.ascontiguousarray(
                w.transpose(0, 2, 1)                    # [L, D, F]
                .reshape(L, KD, 128, KF, 128)
                .transpose(2, 0, 3, 1, 4)               # [128, L, KF, KD, 128]
            )
            .astype(e3m4)
            .view(np.uint8)
        )
        # b_enc pre-scaled by S so acts carry the encode x S factor
        be = np.ascontiguousarray(
            b_enc[:, fs : fs + F].reshape(L, KF, 128).transpose(2, 0, 1)
        ).astype(np.float32) * WD_SCALE
        in_maps.append({"xt": xt, "we": we, "be": be})
    return in_maps


def _shard_wdec(W_dec):
    import ml_dtypes

    e3m4 = np.dtype(ml_dtypes.float8_e3m4)
    shards = []
    for c in range(N_CORES):
        fs = c * F
        # wd[p, h, n, kf, d] = q(S * W_dec[i_n, j_n, fs+kf*128+p, h*384+d])
        wd = np.empty((128, 2, NPAIRS, KF, 384), dtype=np.uint8)
        for n, (i, j) in enumerate(PAIRS):
            blk = np.clip(
                W_dec[i, j, fs : fs + F, :] * WD_SCALE, -WD_MAXN, WD_MAXN
            )  # [F, D]
            wd[:, :, n] = (
                blk.reshape(KF, 128, 2, 384)
                .transpose(1, 2, 0, 3)                  # [128, 2, KF, 384]
                .astype(e3m4)
                .view(np.uint8)
            )
        shards.append(wd)
    return shards


def kernel(x, W_enc, b_enc, b_dec, W_dec, dec_mask=None, **_unused):
    x = np.asarray(x, dtype=np.float32)
    W_enc = np.asarray(W_enc, dtype=np.float32)
    b_enc = np.asarray(b_enc, dtype=np.float32)
    b_dec = np.asarray(b_dec, dtype=np.float32)
    W_dec = np.asarray(W_dec, dtype=np.float32)

    nc = _get_nc()

    in_maps = _shard_inputs(x, W_enc, b_enc)
    wd_shards = _shard_wdec(W_dec)
    for c in range(N_CORES):
        in_maps[c]["wd"] = wd_shards[c]

    res = run_bass_kernel_spmd(nc, in_maps, core_ids=list(range(N_CORES)))

    # host-side all-reduce over feature shards + decoder bias
    recon = np.zeros((L, T, D), dtype=np.float32)
    for c in range(N_CORES):
        recon += np.asarray(res.results[c]["out"]).astype(np.float32)
    recon += b_dec[:, None, :]
    return recon
